# revision 22
# baseline (speedup 1.0000x reference)
"""Trainium2 Bass kernel for nn_Attn_48052094107916 (sparse_attention).

Math (per batch b):
  q = x @ Wq.T -> [N, 4, 16];  k = x @ Wk.T -> [N, 4, 16];  v = x @ Wv.T -> [N, 8, 16]
  attn[g,i,j] = <q[i,g,:], k[j,g,:]>
  mw[i,j,g,l] = (masks @ mask_proj)[i,j,g*8+l]
  scores[l,i,j] = sum_g attn[g,i,j] * mw[i,j,g,l]
  out[i,l,:]  = softmax_j(scores[l,i,:]) @ v[:,l,:]

Key restructuring (24 virtual heads): scores[l] = sum_m masks_m (*) w_{m,l},
w_{m,l} = (P-scaled q) @ k^T computed directly on the TensorEngine.

Score stage: per chunk the three mask products are routed to one of three
engine paths, statically balanced so DVE/Pool/ACT busy times equalize:
  AD: one DVE scalar_tensor_tensor over all 3 psum planes (fused crossing).
  AP: the same op on Pool (gpsimd).
  BC: ACT copies psum->fp16, then 3 DVE tensor_tensor products (2x mode).
The two adds (m1,m2 -> m0) ride HWDGE accumulate DMAs issued from the SP
queue at half-batch granularity; exp on ACT; denominator via a ones column
in the PV stationary; PV psum double-buffered across batches by partition
offset (base 0 / 32) inside the same two psum banks.

Sharding: 8 cores, core r owns query rows [128r, 128r+128) for ALL batches
(sequence parallel).  No collectives.
"""

import os
import sys

import numpy as np

sys.path.insert(0, "/opt/trn_rl_repo")

B, N, C = 8, 1024, 128
G, L, HD = 4, 8, 16
NCORES = 8
RQ = N // NCORES  # query rows per core = 128
NCH = N // 128  # key chunks = 8
HGRP = 4  # chunks per pipeline group (half batch)
NGR = NCH // HGRP  # groups per batch = 2
NSLOT = B * NGR  # 16 pipeline slots

# path of each (b, ch): 'd' = DVE STT (fused psum crossing + product),
# 'c' = ACT copy psum->fp16 + DVE TT products (2x), 'P' = ACT copy + Pool TT.
# GPSIMD cannot touch PSUM, so Pool only gets post-copy SBUF products.
# Counts tuned so DVE/Pool/ACT busy ~equalize.
_PAT = []
for _b in range(B):
    if _b % 2 == 0:
        _PAT.append(["d", "P", "d", "c", "d", "P", "d", "P"])
    else:
        _PAT.append(["d", "P", "d", "d", "d", "P", "d", "c"])
# totals: d=36, P=20, c=8

_cache = {}


def _build():
    import concourse.bacc as bacc
    import concourse.bass as bass
    import concourse.tile as tile
    from concourse import mybir

    f32 = mybir.dt.float32
    bf16 = mybir.dt.bfloat16
    fp16 = mybir.dt.float16
    AF = mybir.ActivationFunctionType
    OP = mybir.AluOpType

    nc = bacc.Bacc("TRN2", target_bir_lowering=False)

    xt_d = nc.dram_tensor("xt", [B, C, N], fp16, kind="ExternalInput")
    xqt_d = nc.dram_tensor("xqt", [B, C, RQ], fp16, kind="ExternalInput")
    mt_d = nc.dram_tensor("maskst", [NCH, 128, 3, 4, 128], fp16,
                          kind="ExternalInput")
    wqt_d = nc.dram_tensor("wqt", [C, 64], fp16, kind="ExternalInput")
    wkt_d = nc.dram_tensor("wkt", [C, 64], fp16, kind="ExternalInput")
    wvt_d = nc.dram_tensor("wvt", [C, C], fp16, kind="ExternalInput")
    pcol_d = nc.dram_tensor("pcol", [64, 3, L], f32, kind="ExternalInput")
    out_d = nc.dram_tensor("out", [B, RQ, C], f32, kind="ExternalOutput")

    with tile.TileContext(nc) as tc, \
            tc.tile_pool(name="singles", bufs=1) as singles, \
            tc.tile_pool(name="xtb", bufs=2) as xtb_pool, \
            tc.tile_pool(name="small", bufs=3) as small, \
            tc.tile_pool(name="cst", bufs=4) as cst_pool, \
            tc.tile_pool(name="scg", bufs=2) as sc_pool, \
            tc.tile_pool(name="pbg", bufs=3) as pb_pool, \
            tc.tile_pool(name="epi", bufs=2) as epi, \
            tc.tile_pool(name="w_ps", bufs=2, space="PSUM") as w_ps_pool, \
            tc.tile_pool(name="pv_ps", bufs=1, space="PSUM") as pv_pool:

        # ---------------- resident tensors ----------------
        wqt = singles.tile([C, 64], fp16)
        wkt = singles.tile([C, 64], fp16)
        wvt = singles.tile([C, C], fp16)
        pcol = singles.tile([64, 3, L], f32)
        nc.sync.dma_start(out=wkt, in_=wkt_d[:, :])

        xqT = singles.tile([C, B, RQ], fp16)
        # masks pre-broadcast over l on host: [j, ch, m, l, i]
        masksT = singles.tile([128, NCH, 3, 4, 128], fp16)
        kT = singles.tile([64, B, N], fp16)
        # P-scaled q, 24 virtual heads; ping-pong 2 batches
        qtb = singles.tile([64, 2, 3, L, RQ], fp16)
        v17 = singles.tile([128, B, NCH, L, 17], bf16)  # [j, ..., l, d|ones]
        nc.gpsimd.memset(v17[:, :, :, :, 16:17], 1.0)

        # PV accumulator: one psum allocation, even batches at partition 0,
        # odd batches at partition 32 (same banks, disjoint partitions).
        pv_ps = pv_pool.tile([49, L, RQ], f32)

        # persistent epilogue staging; rows 17:32 / 49:64 stay zero for the
        # 32x32 block transpose
        pv_sb = singles.tile([64, L, RQ], bf16)
        nc.gpsimd.memset(pv_sb, 0.0)

        # ---------------- per-batch projections ----------------
        def proj_load(b):
            xT = xtb_pool.tile([C, N], fp16, tag="xT", name="xT")
            for h in range(2):
                nc.sync.dma_start(out=xT[:, h * 512:(h + 1) * 512],
                                  in_=xt_d[b, :, h * 512:(h + 1) * 512])
            return xT

        def proj_a(b, xT=None):
            # k/q projections + crossings (ACT)
            if xT is None:
                xT = proj_load(b)
            ps = w_ps_pool.tile([64, 1024], f32, tag="w", name="kps")
            for h in range(2):
                nc.tensor.matmul(ps[:, h * 512:(h + 1) * 512], wkt,
                                 xT[:, h * 512:(h + 1) * 512],
                                 start=True, stop=True)
            nc.scalar.copy(out=kT[:, b, 0:1024], in_=ps)

            ps = w_ps_pool.tile([64, 512], f32, tag="w", name="qps")
            nc.tensor.matmul(ps[:, 0:RQ], wqt, xqT[:, b, :], start=True, stop=True)
            qt_sb = small.tile([64, RQ], fp16, tag="qt", name="qt_sb")
            nc.scalar.copy(out=qt_sb, in_=ps[:, 0:RQ])
            return xT, qt_sb

        def proj_b(b, xT, qt_sb):
            # 24 P-scaled q copies (DVE tensor_scalar, 4x mode) + v17
            for m in range(3):
                for l in range(L):
                    nc.vector.tensor_scalar_mul(
                        qtb[:, b % 2, m, l, :], qt_sb, pcol[:, m, l, None])
            for half in range(2):
                ps = w_ps_pool.tile([128, 4, 128], f32, tag="w", name="vps")
                for c in range(4):
                    ch = half * 4 + c
                    nc.tensor.matmul(ps[:, c],
                                     xT[:, ch * 128:(ch + 1) * 128],
                                     wvt, start=True, stop=True)
                nc.scalar.copy(
                    out=v17[:, b, half * 4:half * 4 + 4, :, 0:16],
                    in_=ps.rearrange("p c (l d) -> p c l d", l=L),
                )

        # ---------------- score stage ----------------
        def products(b, g):
            """PE w-matmuls + per-chunk mask products for one 4-chunk group.
            Returns the sc tile [128, 3, HGRP, L, RQ] (m0 plane = sc[:,0])."""
            sc = sc_pool.tile([128, 3, HGRP, L, RQ], fp16, tag="sc", name="sc")
            for c in range(HGRP):
                ch = g * HGRP + c
                path = _PAT[b][ch]
                kch = kT[:, b, ch * 128:(ch + 1) * 128]
                # [128, 3, 512]; mask value repeated over 4 l's (host
                # broadcast) -- both l-halves read the same 512 pattern
                mkb = masksT[:, ch].rearrange("p m l i -> p m (l i)")
                sc3 = sc[:, :, c].rearrange("p m l i -> p m (l i)")
                cstg = None
                if path in ("c", "P"):
                    cstg = cst_pool.tile([128, 2, 3, 512], fp16, tag="cst",
                                         name="cst")
                for lh in range(2):
                    wp = w_ps_pool.tile([128, 3, 512], f32, tag="w", name="wp")
                    for m in range(3):
                        nc.tensor.matmul(
                            wp[:, m, :],
                            kch,
                            qtb[:, b % 2, m, lh * 4:(lh + 1) * 4, :]
                            .rearrange("p l i -> p (l i)"),
                            start=True, stop=True)
                    out_sl = sc3[:, :, lh * 512:(lh + 1) * 512]
                    mk_sl = mkb
                    if path == "d":
                        nc.vector.scalar_tensor_tensor(
                            out=out_sl, in0=wp, scalar=1.0, in1=mk_sl,
                            op0=OP.mult, op1=OP.mult)
                    elif path == "c":  # ACT copy psum->fp16, DVE 2x products
                        nc.scalar.copy(out=cstg[:, lh], in_=wp)
                        nc.vector.tensor_tensor(
                            out=out_sl, in0=cstg[:, lh], in1=mk_sl, op=OP.mult)
                    else:  # 'P': ACT copy psum->fp16, Pool products
                        nc.scalar.copy(out=cstg[:, lh], in_=wp)
                        nc.gpsimd.tensor_tensor(
                            out=out_sl, in0=cstg[:, lh], in1=mk_sl, op=OP.mult)
            return sc

        def adds(sc):
            # sc[:,0] += sc[:,1]; sc[:,0] += sc[:,2] (SWDGE accumulate DMAs;
            # the adds ride the DMA engines, Pool pays only descriptor-gen).
            # WAW deps on sc[:,0] serialize the two.
            for m in (1, 2):
                if True:  # BISECT: engine adds instead of SWDGE accum
                    nc.vector.tensor_tensor(
                        out=sc[:, 0].rearrange("p c l i -> p (c l i)"),
                        in0=sc[:, 0].rearrange("p c l i -> p (c l i)"),
                        in1=sc[:, m].rearrange("p c l i -> p (c l i)"),
                        op=OP.add)
                else:
                    nc.gpsimd.dma_start(
                        out=sc[:, 0].rearrange("p c l i -> p (c l i)"),
                        in_=sc[:, m].rearrange("p c l i -> p (c l i)"),
                        accum_op=OP.add,
                        max_dma_last_dim=2048)

        def exp_(sc):
            pb = pb_pool.tile([128, HGRP, L, RQ], bf16, tag="pb", name="pb")
            pbf = pb.rearrange("p c l i -> p (c l i)")
            scf = sc[:, 0].rearrange("p c l i -> p (c l i)")
            for h in range(2):
                nc.scalar.activation(out=pbf[:, h * 2048:(h + 1) * 2048],
                                     in_=scf[:, h * 2048:(h + 1) * 2048],
                                     func=AF.Exp)
            return pb

        def pv_mm(b, g, pb):
            base = 0  # BISECT: parity off
            for c in range(HGRP):
                ch = g * HGRP + c
                for l in range(L):
                    nc.tensor.matmul(
                        pv_ps[base:base + 17, l, :],
                        v17[:, b, ch, l, :],
                        pb[:, c, l, :],
                        start=(ch == 0 and l % 4 == 0), stop=(ch == NCH - 1),
                        skip_group_check=True,
                    )

        def epilogue(b):
            base = 0  # BISECT: parity off
            sb = pv_sb[base:base + 17]
            nc.scalar.copy(out=sb, in_=pv_ps[base:base + 17])
            tr = epi.tile([64, L, 4, 32], bf16, tag="pvtr")
            nc.vector.transpose(
                out=tr[base:base + 32].rearrange("p l k r -> p (l k r)"),
                in_=pv_sb[base:base + 32].rearrange("p l i -> p (l i)"),
            )
            denr = epi.tile([64, L, 4], f32, tag="denr")
            nc.vector.reciprocal(out=denr[base:base + 32],
                                 in_=tr[base:base + 32, :, :, 16])
            ob = epi.tile([64, L, 4, 16], f32, tag="ob")
            nc.vector.tensor_tensor(
                out=ob[base:base + 32],
                in0=tr[base:base + 32, :, :, 0:16],
                in1=denr[base:base + 32, :, :, None].to_broadcast((32, L, 4, 16)),
                op=OP.mult,
            )
            # out[b, kb*32+r, l*16+d] <- ob[base+r, l, kb, d]
            ob_dst = bass.AP(
                tensor=out_d, offset=b * RQ * C,
                ap=[[C, 32], [16, L], [32 * C, 4], [1, 16]],
            )
            nc.sync.dma_start(out=ob_dst, in_=ob[base:base + 32])

        # ---------------- schedule ----------------
        xTpre = proj_load(0)
        nc.sync.dma_start(out=wqt, in_=wqt_d[:, :])
        nc.sync.dma_start(out=wvt, in_=wvt_d[:, :])
        nc.sync.dma_start(out=pcol, in_=pcol_d[:, :, :])
        for bb in range(B):
            nc.sync.dma_start(out=xqT[:, bb], in_=xqt_d[bb])
        xT0, qt0 = proj_a(0, xTpre)
        proj_b(0, xT0, qt0)
        for ch in range(NCH):
            nc.sync.dma_start(out=masksT[:, ch], in_=mt_d[ch])

        slots = []  # (b, g, sc, pb-or-None) pipeline
        carry = None

        for s in range(NSLOT):
            b, g = divmod(s, NGR)
            # stage 3: PV for slot s-3 (+ epilogue at batch end)
            if s >= 3:
                sb_, sg_, ssc, spb = slots[s - 3]
                pv_mm(sb_, sg_, spb)
                if sg_ == NGR - 1:
                    epilogue(sb_)
            # stage 2: exp for slot s-2
            if s >= 2:
                sb_, sg_, ssc, _ = slots[s - 2]
                slots[s - 2] = (sb_, sg_, ssc, exp_(ssc))
            # interleave next batch's projections
            if b + 1 < B:
                if g == 0:
                    carry = proj_a(b + 1)
                else:
                    proj_b(b + 1, *carry)
            # stage 0: this slot's products
            sc = products(b, g)
            slots.append((b, g, sc, None))
            # stage 1: adds for slot s-1
            if s >= 1:
                adds(slots[s - 1][2])

        # drain
        adds(slots[NSLOT - 1][2])
        for s in (NSLOT - 2, NSLOT - 1):
            sb_, sg_, ssc, _ = slots[s]
            slots[s] = (sb_, sg_, ssc, exp_(ssc))
        for s in (NSLOT - 3, NSLOT - 2, NSLOT - 1):
            sb_, sg_, ssc, spb = slots[s]
            pv_mm(sb_, sg_, spb)
            if sg_ == NGR - 1:
                epilogue(sb_)

    nc.compile()
    return nc


def _get_graph():
    if "nc" not in _cache:
        _cache["nc"] = _build()
    return _cache["nc"]


def kernel(x, masks, Wq, Wk, Wv, mask_proj):
    from concourse import bass_utils

    x = np.asarray(x, dtype=np.float32)
    masks = np.asarray(masks, dtype=np.float32)
    Wq = np.asarray(Wq, dtype=np.float32)
    Wk = np.asarray(Wk, dtype=np.float32)
    Wv = np.asarray(Wv, dtype=np.float32)
    mask_proj = np.asarray(mask_proj, dtype=np.float32)

    f16 = np.float16
    xt = np.ascontiguousarray(x.transpose(0, 2, 1)).astype(f16)  # [B, C, N]
    wqt = np.ascontiguousarray(Wq.T).astype(f16)
    wkt = np.ascontiguousarray(Wk.T).astype(f16)
    wvt = np.ascontiguousarray(Wv.T).astype(f16)
    # pcol[gd, m, l] = mask_proj[m, g(gd)*L + l]
    g_of = (np.arange(64) // HD)
    pcol = np.ascontiguousarray(
        mask_proj[None, :, :].repeat(64, 0)[
            np.arange(64)[:, None, None],
            np.arange(3)[None, :, None],
            (g_of[:, None, None] * L + np.arange(L)[None, None, :])]
    ).astype(np.float32)

    in_maps = []
    for r in range(NCORES):
        sl = slice(r * RQ, (r + 1) * RQ)
        # maskst[ch, j, m, l, i] = masks[r*128+i, ch*128+j, m] (bcast over l)
        msl = masks[sl]  # [i=128, N, 3]
        mt = np.ascontiguousarray(
            msl.reshape(RQ, NCH, 128, 3).transpose(1, 2, 3, 0)[:, :, :, None, :]
            .repeat(4, axis=3)).astype(f16)
        in_maps.append({
            "xt": xt,
            "xqt": np.ascontiguousarray(xt[:, :, sl]),
            "maskst": mt,
            "wqt": wqt, "wkt": wkt, "wvt": wvt, "pcol": pcol,
        })

    nc = _get_graph()
    trace = bool(int(os.environ.get("KBENCH_TRACE", "0")))
    try:
        res = bass_utils.run_bass_kernel_spmd(
            nc, in_maps, core_ids=list(range(NCORES)), trace=trace,
        )
    except (ImportError, ModuleNotFoundError):
        res = bass_utils.run_bass_kernel_spmd(
            nc, in_maps, core_ids=list(range(NCORES)), trace=False,
        )
    _cache["last_exec_time_ns"] = getattr(res, "exec_time_ns", None)

    out = np.empty((B, N, C), dtype=np.float32)
    for r in range(NCORES):
        out[:, r * RQ:(r + 1) * RQ, :] = res.results[r]["out"]
    return out
